# revision 9
# baseline (speedup 1.0000x reference)
"""ChannelRoll Trainium2 Bass kernel.

out[b,h,w,c] = x[b,h,w,(c + shift_map[b,h,w,0]) % 256]

Strategy (pure data-parallel over batch, 8 cores):
  - Each core gets 4 batches = 12544 rows of 256 fp32 channels.
  - Rows are assigned to SBUF partitions so that each partition owns T
    consecutive rows (contiguous stores).
  - For each row r with shift m, one indirect (gather) DMA reads the
    512-element window x_flat[r*256 + m - 256 : r*256 + m + 256].
    The window's upper half V[256:512] holds the un-wrapped part of the
    rolled row, the lower half V[0:256] holds the wrapped part; a
    per-partition select between the two halves (threshold 256-m along
    the channel axis) produces the rolled row exactly.
  - x is padded with one zero row at each end so every row's window
    stays in bounds (front pad makes row 0's start m >= 0).
"""

import numpy as np

B, H, W, C = 32, 56, 56, 256
NCORES = 8
P = 128
RC = (B // NCORES) * H * W  # rows per core = 12544
COLS = RC // P  # 98 row-columns per partition
T = 7  # rows per partition per super-tile
S = COLS // T  # 14 super-tiles
PAD_ELEMS = (RC + 2) * C  # front + back zero-row padded flat x length


def _build(tc, out_ap, x_ap, shift_ap, n_super=S, rows_per_part=T):
    """Emit the kernel body. out: [RC, C] f32, x: [PAD_ELEMS, 1] f32,
    shift: [P, COLS] int32 pre-permuted so [p, s*T+t] = m of row
    s*(P*T) + p*T + t."""
    import concourse.bass as bass
    import concourse.mybir as mybir

    nc = tc.nc
    cols = n_super * rows_per_part
    rows_per_super = P * rows_per_part

    # out viewed [p, s, (t c)]: row s*(P*T) + p*T + t -> partition p
    out_v = out_ap.rearrange("(s p t) c -> p s (t c)", s=n_super, p=P, t=rows_per_part)

    with tc.tile_pool(name="const", bufs=1) as cpool:
        # iota over channel index, repeated per row-in-tile: [P, T*C] i16
        iota_c = cpool.tile([P, rows_per_part * C], mybir.dt.int16)
        nc.gpsimd.iota(
            iota_c[:].rearrange("p (t c) -> p t c", c=C),
            pattern=[[0, rows_per_part], [1, C]],
            base=0,
            channel_multiplier=0,
        )
        # rowbase = row index in ROW units (iota pattern steps must fit i16)
        rowbase = cpool.tile([P, cols], mybir.dt.int32)
        nc.gpsimd.iota(
            rowbase[:].rearrange("p (s t) -> p s t", t=rows_per_part),
            pattern=[[rows_per_super, n_super], [1, rows_per_part]],
            base=0,
            channel_multiplier=rows_per_part,
        )
        m_sb = cpool.tile([P, cols], mybir.dt.int32)
        nc.sync.dma_start(out=m_sb[:], in_=shift_ap)

        # window start into front-padded x: row*C + m  (>= 0 always)
        idx = cpool.tile([P, cols], mybir.dt.int32)
        nc.vector.scalar_tensor_tensor(
            out=idx[:],
            in0=rowbase[:],
            scalar=C,
            in1=m_sb[:],
            op0=mybir.AluOpType.mult,
            op1=mybir.AluOpType.add,
        )
        # thresh = C - m ; i16 for the mask compare
        thresh32 = cpool.tile([P, cols], mybir.dt.int32)
        nc.vector.tensor_scalar(
            out=thresh32[:],
            in0=m_sb[:],
            scalar1=-1,
            scalar2=C,
            op0=mybir.AluOpType.mult,
            op1=mybir.AluOpType.add,
        )
        thresh = cpool.tile([P, cols], mybir.dt.int16)
        nc.vector.tensor_copy(out=thresh[:], in_=thresh32[:])

        with tc.tile_pool(name="work", bufs=3) as pool:
            for u in range(n_super):
                csl = slice(u * rows_per_part, (u + 1) * rows_per_part)
                v = pool.tile([P, rows_per_part, 2 * C], mybir.dt.float32)
                # HW DGE consumes exactly one index per partition per
                # indirect DMA -> one gather per row-column.
                for t in range(rows_per_part):
                    col = u * rows_per_part + t
                    nc.gpsimd.indirect_dma_start(
                        out=v[:, t, :],
                        out_offset=None,
                        in_=x_ap,
                        in_offset=bass.IndirectOffsetOnAxis(
                            ap=idx[:, col : col + 1], axis=0
                        ),
                    )
                mask = pool.tile([P, rows_per_part, C], mybir.dt.uint8)
                nc.vector.tensor_tensor(
                    out=mask[:],
                    in0=iota_c[:].rearrange("p (t c) -> p t c", c=C),
                    in1=thresh[:, csl].to_broadcast([P, rows_per_part, C]),
                    op=mybir.AluOpType.is_ge,
                )
                o = pool.tile([P, rows_per_part, C], mybir.dt.float32)
                # no-wrap half, then overwrite wrap positions with wrap half
                nc.vector.tensor_copy(out=o[:], in_=v[:, :, C : 2 * C])
                for t in range(rows_per_part):
                    nc.vector.copy_predicated(
                        o[:, t, :], mask[:, t, :], v[:, t, 0:C]
                    )
                nc.sync.dma_start(
                    out=out_v[:, u, :],
                    in_=o[:].rearrange("p t c -> p (t c)"),
                )


def _shard_inputs(x, shift_map):
    """Full inputs -> per-core (x_pad_flat [PAD_ELEMS,1] f32, shift_perm
    [P, COLS] i32)."""
    x = np.ascontiguousarray(np.asarray(x), dtype=np.float32)
    sm = np.asarray(shift_map).astype(np.int32)
    bpc = B // NCORES
    in_maps = []
    for k in range(NCORES):
        xk = x[k * bpc : (k + 1) * bpc].reshape(RC, C)
        xk_pad = np.concatenate(
            [np.zeros(C, np.float32), xk.reshape(-1), np.zeros(C, np.float32)]
        )
        sk = sm[k * bpc : (k + 1) * bpc].reshape(RC)
        # [p, s*T+t] = m of row s*(P*T) + p*T + t
        sperm = np.ascontiguousarray(
            sk.reshape(S, P, T).transpose(1, 0, 2).reshape(P, COLS)
        )
        in_maps.append(
            {"x_pad": xk_pad.reshape(PAD_ELEMS, 1), "shift_perm": sperm}
        )
    return in_maps


_CACHE = {}


def _get_nc(repeat=1):
    key = ("nc", repeat)
    if key in _CACHE:
        return _CACHE[key]
    import concourse.mybir as mybir
    import concourse.tile as tile
    from concourse import bacc

    nc = bacc.Bacc(
        "TRN2",
        debug=False,
        enable_asserts=False,
        num_devices=NCORES,
    )
    x_d = nc.dram_tensor("x_pad", [PAD_ELEMS, 1], mybir.dt.float32, kind="ExternalInput")
    s_d = nc.dram_tensor("shift_perm", [P, COLS], mybir.dt.int32, kind="ExternalInput")
    o_d = nc.dram_tensor("out", [RC, C], mybir.dt.float32, kind="ExternalOutput")
    with tile.TileContext(nc) as tc:
        for _ in range(repeat):
            _build(tc, o_d.ap(), x_d.ap(), s_d.ap())
    nc.compile()
    _CACHE[key] = nc
    return nc


def kernel(x, shift_map, trace=False):
    from concourse.bass_utils import run_bass_kernel_spmd

    nc = _get_nc()
    in_maps = _shard_inputs(x, shift_map)
    res = run_bass_kernel_spmd(
        nc, in_maps, core_ids=list(range(NCORES)), trace=trace
    )
    bpc = B // NCORES
    out = np.concatenate(
        [r["out"].reshape(bpc, H, W, C) for r in res.results], axis=0
    )
    if trace:
        kernel.last_results = res
    return out


# revision 12
# speedup vs baseline: 1.3581x; 1.3581x over previous
"""ChannelRoll Trainium2 Bass kernel.

out[b,h,w,c] = x[b,h,w,(c + shift_map[b,h,w,0]) % 256]

Strategy (pure data-parallel over batch, 8 cores):
  - Each core gets 4 batches = 12544 rows of 256 fp32 channels.
  - Rows are assigned to SBUF partitions so each partition owns T
    consecutive rows: plain contiguous DMA loads and stores (1.0x
    HBM traffic -- the memory-roofline minimum).
  - The per-row circular roll happens entirely in SBUF with
    gpsimd.local_scatter, whose per-partition independent index
    vectors are exactly the missing "per-row dynamic shift":
    viewing each 256-f32 row as 512 u16, dst[p, (j - 2m) & 511] =
    src[p, j] implements roll-left-by-m with pairs kept intact.
  - Index vectors are two cheap DVE int16 ops per tile.
"""

import numpy as np

B, H, W, C = 32, 56, 56, 256
NCORES = 8
P = 128
RC = (B // NCORES) * H * W  # rows per core = 12544
COLS = RC // P  # 98 row-columns per partition
T = 7  # rows per partition per super-tile
S = COLS // T  # 14 super-tiles
NE = 2 * C  # u16 elements per row = 512


def _build(tc, out_ap, x_ap, shift_ap, n_super=S, rows_per_part=T, merge=3):
    """Emit the kernel body. out/x: [R, C] f32, shift: [P, cols] int32
    pre-permuted so [p, s*T+t] = m of row s*(P*T) + p*T + t.

    merge: rows per local_scatter call (num_elems = merge*512 must stay
    under the 2048-element GPSIMD-RAM limit, so merge <= 3)."""
    import concourse.mybir as mybir

    nc = tc.nc
    cols = n_super * rows_per_part
    assert 1 <= merge <= 3

    # row groups per super-tile, e.g. T=7, merge=3 -> [(0,3),(3,3),(6,1)]
    groups = []
    t0 = 0
    while t0 < rows_per_part:
        g = min(merge, rows_per_part - t0)
        groups.append((t0, g))
        t0 += g

    x_v = x_ap.rearrange("(s p t) c -> p s (t c)", s=n_super, p=P, t=rows_per_part)
    out_v = out_ap.rearrange("(s p t) c -> p s (t c)", s=n_super, p=P, t=rows_per_part)

    with tc.tile_pool(name="const", bufs=1) as cpool:
        # u16-granular column index j = 0..511, same in every partition
        j_iota = cpool.tile([P, NE], mybir.dt.int16)
        nc.gpsimd.iota(j_iota[:], pattern=[[1, NE]], base=0, channel_multiplier=0)
        # toff[t, j] = 512 * (t's position within its merge group)
        toff = cpool.tile([P, rows_per_part, NE], mybir.dt.int16)
        for t0, g in groups:
            nc.gpsimd.iota(
                toff[:, t0 : t0 + g, :],
                pattern=[[NE, g], [0, NE]],
                base=0,
                channel_multiplier=0,
            )
        m_sb = cpool.tile([P, cols], mybir.dt.int32)
        nc.sync.dma_start(out=m_sb[:], in_=shift_ap)
        # 2*m as int16 (u16 units)
        m2 = cpool.tile([P, cols], mybir.dt.int16)
        nc.vector.tensor_scalar(
            out=m2[:],
            in0=m_sb[:],
            scalar1=2,
            scalar2=None,
            op0=mybir.AluOpType.mult,
        )

        with tc.tile_pool(name="work", bufs=3) as pool:
            for u in range(n_super):
                csl = slice(u * rows_per_part, (u + 1) * rows_per_part)
                v = pool.tile([P, rows_per_part, C], mybir.dt.float32)
                nc.sync.dma_start(out=v[:], in_=x_v[:, u, :])
                # idx[p, t, j] = ((j - 2*m[p, u*T+t]) & 511) + toff[t]
                idx = pool.tile([P, rows_per_part, NE], mybir.dt.int16)
                nc.vector.tensor_tensor(
                    out=idx[:],
                    in0=j_iota[:].unsqueeze(1).to_broadcast([P, rows_per_part, NE]),
                    in1=m2[:, csl].to_broadcast([P, rows_per_part, NE]),
                    op=mybir.AluOpType.subtract,
                )
                nc.vector.tensor_scalar(
                    out=idx[:],
                    in0=idx[:],
                    scalar1=NE - 1,
                    scalar2=None,
                    op0=mybir.AluOpType.bitwise_and,
                )
                if merge > 1:
                    nc.vector.tensor_tensor(
                        out=idx[:],
                        in0=idx[:],
                        in1=toff[:],
                        op=mybir.AluOpType.add,
                    )
                o = pool.tile([P, rows_per_part, C], mybir.dt.float32)
                for t0, g in groups:
                    nc.gpsimd.local_scatter(
                        o[:, t0 : t0 + g, :].bitcast(mybir.dt.uint16),
                        v[:, t0 : t0 + g, :].bitcast(mybir.dt.uint16),
                        idx[:, t0 : t0 + g, :],
                        channels=P,
                        num_elems=g * NE,
                        num_idxs=g * NE,
                    )
                nc.sync.dma_start(
                    out=out_v[:, u, :],
                    in_=o[:].rearrange("p t c -> p (t c)"),
                )


def _shard_inputs(x, shift_map):
    """Full inputs -> per-core (x [RC, C] f32, shift_perm [P, COLS] i32)."""
    x = np.ascontiguousarray(np.asarray(x), dtype=np.float32)
    sm = np.asarray(shift_map).astype(np.int32)
    bpc = B // NCORES
    in_maps = []
    for k in range(NCORES):
        xk = np.ascontiguousarray(x[k * bpc : (k + 1) * bpc].reshape(RC, C))
        sk = sm[k * bpc : (k + 1) * bpc].reshape(RC)
        # [p, s*T+t] = m of row s*(P*T) + p*T + t
        sperm = np.ascontiguousarray(
            sk.reshape(S, P, T).transpose(1, 0, 2).reshape(P, COLS)
        )
        in_maps.append({"x": xk, "shift_perm": sperm})
    return in_maps


_CACHE = {}


def _get_nc(repeat=1):
    key = ("nc", repeat)
    if key in _CACHE:
        return _CACHE[key]
    import concourse.mybir as mybir
    import concourse.tile as tile
    from concourse import bacc

    nc = bacc.Bacc(
        "TRN2",
        debug=False,
        enable_asserts=False,
        num_devices=NCORES,
    )
    x_d = nc.dram_tensor("x", [RC, C], mybir.dt.float32, kind="ExternalInput")
    s_d = nc.dram_tensor("shift_perm", [P, COLS], mybir.dt.int32, kind="ExternalInput")
    o_d = nc.dram_tensor("out", [RC, C], mybir.dt.float32, kind="ExternalOutput")
    with tile.TileContext(nc) as tc:
        for _ in range(repeat):
            _build(tc, o_d.ap(), x_d.ap(), s_d.ap())
    nc.compile()
    _CACHE[key] = nc
    return nc


def kernel(x, shift_map, trace=False):
    from concourse.bass_utils import run_bass_kernel_spmd

    nc = _get_nc()
    in_maps = _shard_inputs(x, shift_map)
    res = run_bass_kernel_spmd(
        nc, in_maps, core_ids=list(range(NCORES)), trace=trace
    )
    bpc = B // NCORES
    out = np.concatenate(
        [r["out"].reshape(bpc, H, W, C) for r in res.results], axis=0
    )
    if trace:
        kernel.last_results = res
    return out


# revision 13
# speedup vs baseline: 84.8173x; 62.4511x over previous
"""ChannelRoll Trainium2 Bass kernel.

out[b,h,w,c] = x[b,h,w,(c + shift_map[b,h,w,0]) % 256]

Strategy (pure data-parallel over batch, 8 cores):
  - Each core gets 4 batches = 12544 rows of 256 fp32 channels.
  - Rows are assigned to SBUF partitions so each partition owns T
    consecutive rows: plain contiguous DMA loads and stores (1.0x
    HBM traffic -- the memory-roofline minimum).
  - The per-row circular roll happens entirely in SBUF with
    gpsimd.local_scatter, whose per-partition independent index
    vectors provide the per-row dynamic shift no other engine op has:
    viewing each 256-f32 row as 512 u16, dst[p, (j - 2m) & 511] =
    src[p, j] implements roll-left-by-m with u16 pairs kept intact.
  - merge=3 packs 3 rows into one local_scatter call (GPSIMD-RAM cap
    is 2048 u16) to amortize per-call overhead.
  - Index vectors are three cheap DVE int16 ops per tile.
"""

import numpy as np

B, H, W, C = 32, 56, 56, 256
NCORES = 8
P = 128
RC = (B // NCORES) * H * W  # rows per core = 12544
COLS = RC // P  # 98 row-columns per partition
T = 7  # rows per partition per super-tile
S = COLS // T  # 14 super-tiles
NE = 2 * C  # u16 elements per row = 512
MERGE = 3  # rows per local_scatter call


def _groups(rows_per_part, merge):
    out = []
    t0 = 0
    while t0 < rows_per_part:
        g = min(merge, rows_per_part - t0)
        out.append((t0, g))
        t0 += g
    return out


def _setup(tc, cpool, shift_ap, cols, rows_per_part, merge):
    """Constant tiles: j_iota, toff, m2 (2*m as int16)."""
    import concourse.mybir as mybir

    nc = tc.nc
    j_iota = cpool.tile([P, NE], mybir.dt.int16)
    nc.gpsimd.iota(j_iota[:], pattern=[[1, NE]], base=0, channel_multiplier=0)
    toff = None
    if merge > 1:
        # toff[t, j] = 512 * (t's position within its merge group)
        toff = cpool.tile([P, rows_per_part, NE], mybir.dt.int16)
        for t0, g in _groups(rows_per_part, merge):
            nc.gpsimd.iota(
                toff[:, t0 : t0 + g, :],
                pattern=[[NE, g], [0, NE]],
                base=0,
                channel_multiplier=0,
            )
    m_sb = cpool.tile([P, cols], mybir.dt.int32)
    nc.sync.dma_start(out=m_sb[:], in_=shift_ap)
    m2 = cpool.tile([P, cols], mybir.dt.int16)
    nc.vector.tensor_scalar(
        out=m2[:], in0=m_sb[:], scalar1=2, scalar2=None, op0=mybir.AluOpType.mult
    )
    return {"j_iota": j_iota, "toff": toff, "m2": m2}


def _super_tile(tc, pool, consts, out_v, x_v, u, rows_per_part, merge):
    """Load, roll, store one super-tile (128 partitions x T rows)."""
    import concourse.mybir as mybir

    nc = tc.nc
    j_iota, toff, m2 = consts["j_iota"], consts["toff"], consts["m2"]
    csl = slice(u * rows_per_part, (u + 1) * rows_per_part)

    v = pool.tile([P, rows_per_part, C], mybir.dt.float32)
    nc.sync.dma_start(out=v[:], in_=x_v[:, u, :])
    # idx[p, t, j] = ((j - 2*m[p, u*T+t]) & 511) + toff[t]
    idx = pool.tile([P, rows_per_part, NE], mybir.dt.int16)
    nc.vector.tensor_tensor(
        out=idx[:],
        in0=j_iota[:].unsqueeze(1).to_broadcast([P, rows_per_part, NE]),
        in1=m2[:, csl].to_broadcast([P, rows_per_part, NE]),
        op=mybir.AluOpType.subtract,
    )
    nc.vector.tensor_scalar(
        out=idx[:],
        in0=idx[:],
        scalar1=NE - 1,
        scalar2=None,
        op0=mybir.AluOpType.bitwise_and,
    )
    if merge > 1:
        nc.vector.tensor_tensor(
            out=idx[:], in0=idx[:], in1=toff[:], op=mybir.AluOpType.add
        )
    o = pool.tile([P, rows_per_part, C], mybir.dt.float32)
    for t0, g in _groups(rows_per_part, merge):
        nc.gpsimd.local_scatter(
            o[:, t0 : t0 + g, :].bitcast(mybir.dt.uint16),
            v[:, t0 : t0 + g, :].bitcast(mybir.dt.uint16),
            idx[:, t0 : t0 + g, :],
            channels=P,
            num_elems=g * NE,
            num_idxs=g * NE,
        )
    nc.sync.dma_start(
        out=out_v[:, u, :], in_=o[:].rearrange("p t c -> p (t c)")
    )


def _build(tc, out_ap, x_ap, shift_ap, n_super=S, rows_per_part=T, merge=MERGE):
    """Emit the whole kernel body (setup + all super-tiles)."""
    cols = n_super * rows_per_part
    x_v = x_ap.rearrange("(s p t) c -> p s (t c)", s=n_super, p=P, t=rows_per_part)
    out_v = out_ap.rearrange("(s p t) c -> p s (t c)", s=n_super, p=P, t=rows_per_part)
    with tc.tile_pool(name="const", bufs=1) as cpool:
        consts = _setup(tc, cpool, shift_ap, cols, rows_per_part, merge)
        with tc.tile_pool(name="work", bufs=3) as pool:
            for u in range(n_super):
                _super_tile(tc, pool, consts, out_v, x_v, u, rows_per_part, merge)


def _shard_inputs(x, shift_map):
    """Full inputs -> per-core (x [RC, C] f32, shift_perm [P, COLS] i32)."""
    x = np.ascontiguousarray(np.asarray(x), dtype=np.float32)
    sm = np.asarray(shift_map).astype(np.int32)
    bpc = B // NCORES
    in_maps = []
    for k in range(NCORES):
        xk = np.ascontiguousarray(x[k * bpc : (k + 1) * bpc].reshape(RC, C))
        sk = sm[k * bpc : (k + 1) * bpc].reshape(RC)
        # [p, s*T+t] = m of row s*(P*T) + p*T + t
        sperm = np.ascontiguousarray(
            sk.reshape(S, P, T).transpose(1, 0, 2).reshape(P, COLS)
        )
        in_maps.append({"x": xk, "shift_perm": sperm})
    return in_maps


_CACHE = {}


def _get_nc(repeat=1):
    key = ("nc", repeat)
    if key in _CACHE:
        return _CACHE[key]
    import concourse.mybir as mybir
    import concourse.tile as tile
    from concourse import bacc

    nc = bacc.Bacc(
        "TRN2",
        debug=False,
        enable_asserts=False,
        num_devices=NCORES,
    )
    x_d = nc.dram_tensor("x", [RC, C], mybir.dt.float32, kind="ExternalInput")
    s_d = nc.dram_tensor("shift_perm", [P, COLS], mybir.dt.int32, kind="ExternalInput")
    o_d = nc.dram_tensor("out", [RC, C], mybir.dt.float32, kind="ExternalOutput")
    with tile.TileContext(nc) as tc:
        for _ in range(repeat):
            _build(tc, o_d.ap(), x_d.ap(), s_d.ap())
    nc.compile()
    _CACHE[key] = nc
    return nc


def kernel(x, shift_map, trace=False):
    from concourse.bass_utils import run_bass_kernel_spmd

    nc = _get_nc()
    in_maps = _shard_inputs(x, shift_map)
    res = run_bass_kernel_spmd(
        nc, in_maps, core_ids=list(range(NCORES)), trace=trace
    )
    bpc = B // NCORES
    out = np.concatenate(
        [r["out"].reshape(bpc, H, W, C) for r in res.results], axis=0
    )
    if trace:
        kernel.last_results = res
    return out
